# revision 13
# baseline (speedup 1.0000x reference)
"""Trainium2 Bass kernel for nn_Decoder_14894946583396 (dense_mlp).

Reference computation:
    sized = broadcast(representation[B,1,R] -> [B,S,R])   (ones @ rep)
    h     = relu(sized @ W1^T + b1)                       [B,S,HID]
    out   = h @ W2^T + b2                                 [B,S,OUT]

Because every position s within batch b receives the identical input row
representation[b], the MLP output row is identical for all S positions:
    row[b] = relu(rep[b] @ W1^T + b1) @ W2^T + b2         [B,OUT]
    out[b, s, :] = row[b]  for all s

Sharding: the S axis is degenerate, so the device only computes the
unique rows. OUT columns are sharded 8 ways: every core computes all
B=32 batch rows for its own 128-column slice of the output, writing a
[32,128] f32 shard. The host unshards by concatenating the column
slices and broadcasting the rows across S.

This makes the kernel input-DMA-bound: W1 (replicated, needed in full
by every core because every core computes h for its batches) dominates.
Weights/activations are staged in bf16 (halves DMA bytes; rel-err
~2e-3, far inside the 2e-2 gate); PSUM accumulation stays fp32.

Device pipeline per core (all input DMAs on the sync HWDGE ring, in
transfer order pk, prow, w1 x4 chunks, w2s):
  1. ~3.4 us of dummy matmuls on zeros warm the PE HAM clock gate
     (1.2 -> 2.4 GHz) while the weights stream in.
  2. L1: H[m,h] = x @ W1^T via 8 accumulating matmuls (x^T chunk
     stationary, cheap LDWEIGHTS), each pipelined behind its w1 chunk's
     DMA; bias folded in as a K=1 ones-matmul; relu+bf16-cast on ACT.
  3. H -> H^T via 4 PE transposes (bf16 PSUM), ACT copies to SBUF.
  4. L2: Y[m,o_slice] = H @ W2s^T + b2s, 4 accumulating matmuls + bias.
  5. One 16 KiB output DMA.

Single-sync-wait discipline (walrus rejects 2+ waits per instruction):
the last warmup matmul pre-observes pk's DMA lane so L1's first matmul
only waits w1-chunk-0; transpose #1 pre-observes w2s's lane so L2's
matmuls only wait the ACT copy ticks; a chain of 1-wait SP nops before
the TileContext exit drain leaves the drain with nothing to wait on.
"""

import sys

import numpy as np

if "/opt/trn_rl_repo" not in sys.path:
    sys.path.insert(0, "/opt/trn_rl_repo")

import ml_dtypes

BF16 = ml_dtypes.bfloat16

B, S, R = 32, 1024, 1024
HID, OUT = 512, 1024
N_CORES = 8
OSL = OUT // N_CORES  # output columns per core

RC = R // 128  # layer-1 contraction chunks
HC = HID // 128  # layer-2 contraction chunks
W1_DMA_CHUNKS = 4  # w1 streamed in 4 column-range DMAs (2 rc each)

# pk columns: xT chunks [p, rc*B + m] = rep[m, rc*128+p], then a 32x32
# identity for the PE transposes
XTOFF = 0
IOFF = XTOFF + RC * B
PKW = IOFF + B
# prow columns (single partition row): ones, b1, b2 slice
ONOFF = 0
B1OFF = ONOFF + B
B2OFF = B1OFF + HID
PROWW = B2OFF + OSL

N_WARMUP = 8

_CACHED_NC = None


def _build_nc():
    import concourse.bass as bass
    import concourse.mybir as mybir
    from concourse.tile import TileContext, add_dep_helper

    f32 = mybir.dt.float32
    bf16 = mybir.dt.bfloat16
    relu = mybir.ActivationFunctionType.Relu
    fcopy = mybir.ActivationFunctionType.Copy
    nc = bass.Bass()

    pk = nc.dram_tensor("pk", [128, PKW], bf16, kind="ExternalInput")
    prow = nc.dram_tensor("prow", [1, PROWW], bf16, kind="ExternalInput")
    w1 = nc.dram_tensor("w1", [128, RC * HID], bf16, kind="ExternalInput")
    w2s = nc.dram_tensor("w2s", [128, HC * OSL], bf16, kind="ExternalInput")
    out = nc.dram_tensor("out", [B, OSL], f32, kind="ExternalOutput")

    with TileContext(nc) as tc:
        with (
            tc.tile_pool(name="const", bufs=1) as cpool,
            tc.tile_pool(name="psum_s", bufs=1, space="PSUM") as pp_s,
            tc.tile_pool(name="psum_t", bufs=4, space="PSUM") as pp_t,
            tc.tile_pool(name="psum_y", bufs=1, space="PSUM") as pp_y,
        ):
            # Inputs ride all three dynamic DMA queues in parallel. Descriptor
            # size (per-partition contiguous bytes) dominates HBM-read rate
            # (576B -> ~76 GB/s, 2KB -> ~200, 8KB -> ~260), so w1 streams as
            # two concurrent 4KB/partition DMAs on the two HWDGE rings while
            # the small tensors share the SWDGE ring.
            pk_sb = cpool.tile([128, PKW], bf16, tag="pk")
            d_pk = nc.gpsimd.dma_start(out=pk_sb[:, :], in_=pk[:, :])
            prow_sb = cpool.tile([1, PROWW], bf16, tag="prow")
            d_prow = nc.gpsimd.dma_start(out=prow_sb[0:1, :], in_=prow[0:1, :])
            w1_sb = cpool.tile([128, RC * HID], bf16, tag="w1")
            half = RC * HID // 2
            d_w1 = [
                nc.sync.dma_start(out=w1_sb[:, 0:half], in_=w1[:, 0:half]),
                nc.scalar.dma_start(out=w1_sb[:, half:], in_=w1[:, half:]),
            ]
            w2s_sb = cpool.tile([128, HC * OSL], bf16, tag="w2s")
            d_w2s = nc.gpsimd.dma_start(out=w2s_sb[:, :], in_=w2s[:, :])

            # ---- PE warmup on zeros; shares L1's PSUM tile (a slot handoff
            # would emit a non-elidable same-engine wait) -------------------
            wm_sb = cpool.tile([128, 512], bf16, tag="wm")
            nc.vector.memset(wm_sb[:, :], 0.0)
            ph_full = pp_s.tile([128, HID], f32, tag="s")
            for k in range(N_WARMUP):
                wmm = nc.tensor.matmul(
                    ph_full[:, :],
                    lhsT=wm_sb[:, 0:128],
                    rhs=wm_sb[:, :],
                    start=True,
                    stop=True,
                )
            # the last warmup matmul observes pk's lane so L1's first matmul
            # only needs the w1-chunk-0 wait
            add_dep_helper(wmm.ins, d_pk.ins, sync=True, reason="observe pk")

            # ---- L1: H[m, h] = x @ W1^T + b1, relu -------------------------
            ph = ph_full[0:B, :]
            for rc in range(RC):
                nc.tensor.matmul(
                    ph[:, :],
                    lhsT=pk_sb[:, XTOFF + rc * B : XTOFF + (rc + 1) * B],
                    rhs=w1_sb[:, rc * HID : rc * HID + HID],
                    start=(rc == 0),
                    stop=False,
                )
            nc.tensor.matmul(
                ph[:, :],
                lhsT=prow_sb[0:1, ONOFF : ONOFF + B],
                rhs=prow_sb[0:1, B1OFF : B1OFF + HID],
                start=False,
                stop=True,
            )
            # relu in two DVE passes so the first transposes can start after
            # the first half lands (Tile serializes cross-engine readers of
            # one PSUM tile, so both halves stay on DVE)
            h_a = cpool.tile([B, HID // 2], bf16, tag="ha")
            h_b = cpool.tile([B, HID // 2], bf16, tag="hb")
            nc.vector.tensor_scalar_max(h_a[:, :], ph[:, 0 : HID // 2], 0.0)
            nc.vector.tensor_scalar_max(h_b[:, :], ph[:, HID // 2 : HID], 0.0)

            # ---- H -> H^T (stationary operand for L2), bf16 PE transposes;
            # PSUM->SBUF copies alternate DVE/ACT so they pipeline 2-wide ----
            ht_sb = cpool.tile([128, HC * B], bf16, tag="ht")
            for hc in range(HC):
                h_half = h_a if hc < HC // 2 else h_b
                hoff = (hc % (HC // 2)) * 128
                pt = pp_t.tile([128, B], bf16, tag="t")
                tmm = nc.tensor.transpose(
                    pt[:, :],
                    h_half[0:B, hoff : hoff + 128],
                    pk_sb[0:B, IOFF : IOFF + B],
                )
                if hc == 1:
                    # free wait slot: pre-observe w2s's lane for L2
                    add_dep_helper(tmm.ins, d_w2s.ins, sync=True, reason="observe w2s")
                dst = ht_sb[:, hc * B : (hc + 1) * B]
                if hc % 2 == 0:
                    nc.vector.tensor_copy(dst, pt[:, :])
                else:
                    last_act = nc.scalar.activation(dst, pt[:, :], fcopy)

            # ---- L2: Y[m, o_slice] = H @ W2s^T + b2s -----------------------
            py = pp_y.tile([B, OSL], f32, tag="y")
            for hc in range(HC):
                nc.tensor.matmul(
                    py[:, :],
                    lhsT=ht_sb[:, hc * B : (hc + 1) * B],
                    rhs=w2s_sb[:, hc * OSL : (hc + 1) * OSL],
                    start=(hc == 0),
                    stop=False,
                )
            last_mm = nc.tensor.matmul(
                py[:, :],
                lhsT=prow_sb[0:1, ONOFF : ONOFF + B],
                rhs=prow_sb[0:1, B2OFF : B2OFF + OSL],
                start=False,
                stop=True,
            )
            o_sb = cpool.tile([B, OSL], f32, tag="o")
            last_dve = nc.vector.tensor_copy(o_sb[:, :], py[:, :])
            d_out = nc.sync.dma_start(out=out[:, :], in_=o_sb[:, :])

            # The kernel-tail drain waits on every proc's final tick, but this
            # walrus allows at most ONE sync wait per instruction. Chain SP
            # nops, one dependency each, so SP's vector clock observes the
            # final tick of every DMA lane and engine before the drain.
            tail = [d_out, d_pk, d_prow] + d_w1 + [d_w2s, last_mm, last_act, last_dve]
            for d in tail:
                n = nc.sync.nop(nofuse=True)
                add_dep_helper(
                    n.ins, d.ins, sync=True, reason="observe final ticks pre-drain"
                )

    return nc


def _get_nc():
    global _CACHED_NC
    if _CACHED_NC is None:
        _CACHED_NC = _build_nc()
    return _CACHED_NC


def _prep_in_maps(representation, W1, b1, W2, b2):
    rep = np.asarray(representation, dtype=np.float32).reshape(B, R)
    w1 = np.asarray(W1, dtype=np.float32)
    w2 = np.asarray(W2, dtype=np.float32)
    b1 = np.asarray(b1, dtype=np.float32)
    b2 = np.asarray(b2, dtype=np.float32)

    # pk: xT chunks + 32x32 identity (identical for every core)
    pk = np.zeros((128, PKW), dtype=np.float32)
    xt = rep.T  # [R, B]
    pk[:, XTOFF : XTOFF + RC * B] = (
        xt.reshape(RC, 128, B).transpose(1, 0, 2).reshape(128, RC * B)
    )
    pk[0:B, IOFF : IOFF + B] = np.eye(B, dtype=np.float32)
    pk = pk.astype(BF16)

    # w1p[p, rc*HID + h] = W1[h, rc*128+p]
    w1p = np.ascontiguousarray(
        w1.T.reshape(RC, 128, HID).transpose(1, 0, 2).reshape(128, RC * HID)
    ).astype(BF16)

    in_maps = []
    for c in range(N_CORES):
        sl = slice(c * OSL, (c + 1) * OSL)
        prow = np.zeros((1, PROWW), dtype=np.float32)
        prow[0, ONOFF : ONOFF + B] = 1.0
        prow[0, B1OFF : B1OFF + HID] = b1
        prow[0, B2OFF : B2OFF + OSL] = b2[sl]
        # w2sp[p, hc*OSL + o] = W2[c*OSL+o, hc*128+p]
        w2sl = w2[sl]  # [OSL, HID]
        w2sp = np.ascontiguousarray(
            w2sl.T.reshape(HC, 128, OSL).transpose(1, 0, 2).reshape(128, HC * OSL)
        ).astype(BF16)
        in_maps.append(
            {"pk": pk, "prow": prow.astype(BF16), "w1": w1p, "w2s": w2sp}
        )
    return in_maps


def run_sharded(representation, W1, b1, W2, b2, **run_kwargs):
    """Compile+run on 8 cores; returns (full_output, BassKernelResults)."""
    from concourse.bass_utils import run_bass_kernel_spmd

    nc = _get_nc()
    in_maps = _prep_in_maps(representation, W1, b1, W2, b2)
    res = run_bass_kernel_spmd(nc, in_maps, core_ids=list(range(N_CORES)), **run_kwargs)
    rows = np.concatenate([r["out"] for r in res.results], axis=1)  # [B, OUT]
    full = np.ascontiguousarray(
        np.broadcast_to(rows[:, None, :], (B, S, OUT))
    )
    return full, res


def kernel(representation, size_matrix=None, W1=None, b1=None, W2=None, b2=None):
    # size_matrix only contributes its shape in the reference (ones_like);
    # its values are unused.
    full, _ = run_sharded(representation, W1, b1, W2, b2)
    return full


# revision 21
# speedup vs baseline: 1.1161x; 1.1161x over previous
"""Trainium2 Bass kernel for nn_Decoder_14894946583396 (dense_mlp).

Reference computation:
    sized = broadcast(representation[B,1,R] -> [B,S,R])   (ones @ rep)
    h     = relu(sized @ W1^T + b1)                       [B,S,HID]
    out   = h @ W2^T + b2                                 [B,S,OUT]

Because every position s within batch b receives the identical input row
representation[b], the MLP output row is identical for all S positions:
    row[b] = relu(rep[b] @ W1^T + b1) @ W2^T + b2         [B,OUT]
    out[b, s, :] = row[b]  for all s

Sharding: the S axis is degenerate, so the device only computes the
unique rows. OUT columns are sharded 8 ways: every core computes all
B=32 batch rows for its own 128-column slice of the output, writing a
[32,128] f32 shard. The host unshards by concatenating the column
slices and broadcasting the rows across S.

This makes the kernel input-DMA-bound: W1 (replicated, needed in full
by every core because every core computes h for its batches) dominates.
Weights/activations are staged in bf16 (halves DMA bytes; rel-err
~2e-3, far inside the 2e-2 gate); PSUM accumulation stays fp32.

Device pipeline per core (all input DMAs on the sync HWDGE ring, in
transfer order pk, prow, w1 x4 chunks, w2s):
  1. ~3.4 us of dummy matmuls on zeros warm the PE HAM clock gate
     (1.2 -> 2.4 GHz) while the weights stream in.
  2. L1: H[m,h] = x @ W1^T via 8 accumulating matmuls (x^T chunk
     stationary, cheap LDWEIGHTS), each pipelined behind its w1 chunk's
     DMA; bias folded in as a K=1 ones-matmul; relu+bf16-cast on ACT.
  3. H -> H^T via 4 PE transposes (bf16 PSUM), ACT copies to SBUF.
  4. L2: Y[m,o_slice] = H @ W2s^T + b2s, 4 accumulating matmuls + bias.
  5. One 16 KiB output DMA.

Single-sync-wait discipline (walrus rejects 2+ waits per instruction):
the last warmup matmul pre-observes pk's DMA lane so L1's first matmul
only waits w1-chunk-0; transpose #1 pre-observes w2s's lane so L2's
matmuls only wait the ACT copy ticks; a chain of 1-wait SP nops before
the TileContext exit drain leaves the drain with nothing to wait on.
"""

import sys

import numpy as np

if "/opt/trn_rl_repo" not in sys.path:
    sys.path.insert(0, "/opt/trn_rl_repo")

import ml_dtypes

BF16 = ml_dtypes.bfloat16

B, S, R = 32, 1024, 1024
HID, OUT = 512, 1024
N_CORES = 8
OSL = OUT // N_CORES  # output columns per core

RC = R // 128  # layer-1 contraction chunks
HC = HID // 128  # layer-2 contraction chunks
W1_DMA_CHUNKS = 4  # w1 streamed in 4 column-range DMAs (2 rc each)

# pk columns: xT chunks [p, rc*B + m] = rep[m, rc*128+p], then a 32x32
# identity for the PE transposes
XTOFF = 0
IOFF = XTOFF + RC * B
PKW = IOFF + B
# prow columns (single partition row): ones, b1, b2 slice
ONOFF = 0
B1OFF = ONOFF + B
B2OFF = B1OFF + HID
PROWW = B2OFF + OSL

N_WARMUP = 7

_CACHED_NC = None


def _build_nc():
    import concourse.bass as bass
    import concourse.mybir as mybir
    from concourse.tile import TileContext, add_dep_helper

    f32 = mybir.dt.float32
    bf16 = mybir.dt.bfloat16
    relu = mybir.ActivationFunctionType.Relu
    fcopy = mybir.ActivationFunctionType.Copy
    nc = bass.Bass()

    pk = nc.dram_tensor("pk", [128, PKW], bf16, kind="ExternalInput")
    prow = nc.dram_tensor("prow", [1, PROWW], bf16, kind="ExternalInput")
    w1 = nc.dram_tensor("w1", [128, RC * HID], bf16, kind="ExternalInput")
    w2s = nc.dram_tensor("w2s", [128, HC * OSL], bf16, kind="ExternalInput")
    out = nc.dram_tensor("out", [B, OSL], f32, kind="ExternalOutput")

    with TileContext(nc) as tc:
        with (
            tc.tile_pool(name="const", bufs=1) as cpool,
            tc.tile_pool(name="psum_s", bufs=1, space="PSUM") as pp_s,
            tc.tile_pool(name="psum_t", bufs=2, space="PSUM") as pp_t,
            tc.tile_pool(name="psum_y", bufs=1, space="PSUM") as pp_y,
        ):
            # DMA layout notes (HW-measured): per-partition descriptor size
            # dominates HBM-read rate (576B -> ~76 GB/s, 2KB -> ~200, 4KB ->
            # ~220, 8KB -> ~260); extra queues do NOT add aggregate bandwidth
            # (8 cores share HBM), and the SWDGE (gpsimd) queue has ~3-4us
            # first-data latency, so everything rides the two HWDGE rings:
            # sync carries w1 (two h-column halves so layer-1's first half
            # computes under the second half's stream), scalar carries the
            # small tensors.
            pk_sb = cpool.tile([128, PKW], bf16, tag="pk")
            d_pk = nc.scalar.dma_start(out=pk_sb[:, :], in_=pk[:, :])
            prow_sb = cpool.tile([1, PROWW], bf16, tag="prow")
            d_prow = nc.scalar.dma_start(out=prow_sb[0:1, :], in_=prow[0:1, :])
            w1_sb = cpool.tile([128, RC * HID], bf16, tag="w1")
            half = RC * HID // 2
            d_w1 = [
                nc.sync.dma_start(out=w1_sb[:, 0:half], in_=w1[:, 0:half]),
                nc.sync.dma_start(out=w1_sb[:, half:], in_=w1[:, half:]),
            ]
            w2s_sb = cpool.tile([128, HC * OSL], bf16, tag="w2s")
            d_w2s = nc.scalar.dma_start(out=w2s_sb[:, :], in_=w2s[:, :])

            # ---- PE warmup on zeros; shares L1's PSUM tile (a slot handoff
            # would emit a non-elidable same-engine wait) -------------------
            wm_sb = cpool.tile([128, 512], bf16, tag="wm")
            nc.vector.memset(wm_sb[:, :], 0.0)
            ph_full = pp_s.tile([128, HID], f32, tag="s")
            for k in range(N_WARMUP):
                wmm = nc.tensor.matmul(
                    ph_full[:, :],
                    lhsT=wm_sb[:, 0:128],
                    rhs=wm_sb[:, :],
                    start=True,
                    stop=True,
                )
            # the last warmup matmul observes pk's lane so L1's first matmul
            # only needs the w1-chunk-0 wait
            add_dep_helper(wmm.ins, d_pk.ins, sync=True, reason="observe pk")

            # ---- Two half-pipelines over h-columns. w1 is packed so half g
            # holds W1 rows [g*256, (g+1)*256) for every rc chunk; half 0's
            # L1 matmuls, relu, transposes and first two L2 accumulations all
            # run while half 1 is still streaming. --------------------------
            # separate PSUM tiles per half — co-readers/WAR on one shared
            # PSUM tile get serialized by Tile with non-elidable waits
            HH = HID // 2
            ht_sb = cpool.tile([128, HC * B], bf16, tag="ht")
            py = pp_y.tile([B, OSL], f32, tag="y")
            ph_a = pp_s.tile([B, HH], f32, tag="pha")
            ph_b = pp_s.tile([B, HH], f32, tag="phb")
            ph_halves = [ph_a, ph_b]
            for g in range(2):
                ph_g = ph_halves[g]
                for rc in range(RC):
                    nc.tensor.matmul(
                        ph_g[:, :],
                        lhsT=pk_sb[:, XTOFF + rc * B : XTOFF + (rc + 1) * B],
                        rhs=w1_sb[:, g * half + rc * HH : g * half + (rc + 1) * HH],
                        start=(rc == 0),
                        stop=False,
                    )
                nc.tensor.matmul(
                    ph_g[:, :],
                    lhsT=prow_sb[0:1, ONOFF : ONOFF + B],
                    rhs=prow_sb[0:1, B1OFF + g * HH : B1OFF + (g + 1) * HH],
                    start=False,
                    stop=True,
                )
                h_g = cpool.tile([B, HH], bf16, tag=f"h{g}")
                nc.vector.tensor_scalar_max(h_g[:, :], ph_g[:, :], 0.0)
                for j in range(2):
                    hc = g * 2 + j
                    pt = pp_t.tile([128, B], bf16, tag="t")
                    tmm = nc.tensor.transpose(
                        pt[:, :],
                        h_g[0:B, j * 128 : (j + 1) * 128],
                        pk_sb[0:B, IOFF : IOFF + B],
                    )
                    if hc == 1:
                        # free wait slot: pre-observe w2s's lane for L2
                        add_dep_helper(
                            tmm.ins, d_w2s.ins, sync=True, reason="observe w2s"
                        )
                    dst = ht_sb[:, hc * B : (hc + 1) * B]
                    if g == 0:
                        nc.vector.tensor_copy(dst, pt[:, :])
                    else:
                        last_act = nc.scalar.activation(dst, pt[:, :], fcopy)
                for j in range(2):
                    hc = g * 2 + j
                    nc.tensor.matmul(
                        py[:, :],
                        lhsT=ht_sb[:, hc * B : (hc + 1) * B],
                        rhs=w2s_sb[:, hc * OSL : (hc + 1) * OSL],
                        start=(hc == 0),
                        stop=False,
                    )
            last_mm = nc.tensor.matmul(
                py[:, :],
                lhsT=prow_sb[0:1, ONOFF : ONOFF + B],
                rhs=prow_sb[0:1, B2OFF : B2OFF + OSL],
                start=False,
                stop=True,
            )
            o_sb = cpool.tile([B, OSL], f32, tag="o")
            last_dve = nc.vector.tensor_copy(o_sb[:, :], py[:, :])
            d_out = nc.sync.dma_start(out=out[:, :], in_=o_sb[:, :])

            # The kernel-tail drain waits on every proc's final tick, but this
            # walrus allows at most ONE sync wait per instruction. Chain SP
            # nops, one dependency each, so SP's vector clock observes the
            # final tick of every DMA lane and engine before the drain.
            tail = [d_out, d_pk, d_prow] + d_w1 + [d_w2s, last_mm, last_act, last_dve]
            for d in tail:
                n = nc.sync.nop(nofuse=True)
                add_dep_helper(
                    n.ins, d.ins, sync=True, reason="observe final ticks pre-drain"
                )

    return nc


def _get_nc():
    global _CACHED_NC
    if _CACHED_NC is None:
        _CACHED_NC = _build_nc()
    return _CACHED_NC


def _prep_in_maps(representation, W1, b1, W2, b2):
    rep = np.asarray(representation, dtype=np.float32).reshape(B, R)
    w1 = np.asarray(W1, dtype=np.float32)
    w2 = np.asarray(W2, dtype=np.float32)
    b1 = np.asarray(b1, dtype=np.float32)
    b2 = np.asarray(b2, dtype=np.float32)

    # pk: xT chunks + 32x32 identity (identical for every core)
    pk = np.zeros((128, PKW), dtype=np.float32)
    xt = rep.T  # [R, B]
    pk[:, XTOFF : XTOFF + RC * B] = (
        xt.reshape(RC, 128, B).transpose(1, 0, 2).reshape(128, RC * B)
    )
    pk[0:B, IOFF : IOFF + B] = np.eye(B, dtype=np.float32)
    pk = pk.astype(BF16)

    # w1p[p, g*2048 + rc*256 + h'] = W1[g*256 + h', rc*128+p] — h-half-major
    # so each 512KB DMA half covers all rc chunks for 256 h-columns
    w1p = np.ascontiguousarray(
        w1.T.reshape(RC, 128, 2, HID // 2)
        .transpose(1, 2, 0, 3)
        .reshape(128, RC * HID)
    ).astype(BF16)

    in_maps = []
    for c in range(N_CORES):
        sl = slice(c * OSL, (c + 1) * OSL)
        prow = np.zeros((1, PROWW), dtype=np.float32)
        prow[0, ONOFF : ONOFF + B] = 1.0
        prow[0, B1OFF : B1OFF + HID] = b1
        prow[0, B2OFF : B2OFF + OSL] = b2[sl]
        # w2sp[p, hc*OSL + o] = W2[c*OSL+o, hc*128+p]
        w2sl = w2[sl]  # [OSL, HID]
        w2sp = np.ascontiguousarray(
            w2sl.T.reshape(HC, 128, OSL).transpose(1, 0, 2).reshape(128, HC * OSL)
        ).astype(BF16)
        in_maps.append(
            {"pk": pk, "prow": prow.astype(BF16), "w1": w1p, "w2s": w2sp}
        )
    return in_maps


def run_sharded(representation, W1, b1, W2, b2, **run_kwargs):
    """Compile+run on 8 cores; returns (full_output, BassKernelResults)."""
    from concourse.bass_utils import run_bass_kernel_spmd

    nc = _get_nc()
    in_maps = _prep_in_maps(representation, W1, b1, W2, b2)
    res = run_bass_kernel_spmd(nc, in_maps, core_ids=list(range(N_CORES)), **run_kwargs)
    rows = np.concatenate([r["out"] for r in res.results], axis=1)  # [B, OUT]
    full = np.ascontiguousarray(
        np.broadcast_to(rows[:, None, :], (B, S, OUT))
    )
    return full, res


def kernel(representation, size_matrix=None, W1=None, b1=None, W2=None, b2=None):
    # size_matrix only contributes its shape in the reference (ones_like);
    # its values are unused.
    full, _ = run_sharded(representation, W1, b1, W2, b2)
    return full
